# revision 1
# baseline (speedup 1.0000x reference)
"""Trainium2 Bass kernel for nn_AttnResBlockUp (B=16, IN=512, OUT=256, H=W=32, L=32).

Sharding: data-parallel over batch (2 items per core, 8 cores); BatchNorm
batch stats sync'd with a cross-core AllReduce (sync-BN). Everything else is
batch-independent.

Structure (per core):
  pass0   stream x chunks: BN1 partial sums (reduce + ACT Square accum)
  AR1     AllReduce [sum, sumsq] (4 KB) -> bn1 scale/shift
  stage1  per 512-col chunk: img1 mm, cosine-sim attention (norms folded as
          broadcast multiplies), ctx mm, gamma/beta mm, fused BN-affine+relu
          into a zero-padded 34x34 tile; shortcut 1x1 mm on the same chunks
  conv1   4 subpixel 2x2 convs (host-folded weights) == 3x3 conv on up2(x);
          output chunks spilled to DRAM (f32r) with fused BN2 partial stats
  AR2     AllReduce -> bn2 scale/shift
  stage2  same as stage1 at 64x64 from spilled out1 chunks, into 66x66 pads
  conv2   9-tap 3x3 conv + broadcast-add of up2(shortcut) -> store

Matmuls run as float32r (full-rate fp32 mode, ~1e-4 rel err); elementwise fp32.
float32r operands must be produced by an on-chip rounding op (DVE/ACT), so
DMA-loaded weights/x pass through a cast; the DRAM spill stays f32r.
"""
import sys
sys.path.insert(0, "/opt/trn_rl_repo")

import contextlib
import numpy as np
import concourse.bass as bass
import concourse.bacc as bacc
import concourse.mybir as mybir
import concourse.tile as tile

F32 = mybir.dt.float32
F32R = mybir.dt.float32r
AX = mybir.AxisListType
OP = mybir.AluOpType
ACT = mybir.ActivationFunctionType

B, IN, OUT, GD, TD, H, W, L = 16, 512, 256, 256, 256, 32, 32, 32
COND = GD + TD
EPS_BN = 1e-5
N_CORES = 8
B_LOC = B // N_CORES
P1 = H * W                    # 1024
P2 = 4 * P1                   # 4096
CHUNK = 512
SPILL_F32R_DIRECT = True      # DMA-read spilled f32r directly into matmuls


def build_program(num_devices=N_CORES, use_collectives=True):
    nc = bacc.Bacc("TRN2", target_bir_lowering=False, debug=False,
                   num_devices=num_devices)

    def din(name, shape):
        return nc.dram_tensor(name, list(shape), F32, kind="ExternalInput")

    x_d = din("x", (B_LOC, IN, P1))
    gc_d = din("gcT", (GD, B_LOC))
    words_d = din("words", (B_LOC, TD, L))
    wordsT_d = din("wordsT", (B_LOC, L, TD))
    mask_d = din("mask01", (B_LOC, L))
    wimg1_d = din("w_img1T", (IN, TD))
    wimg2_d = din("w_img2T", (OUT, TD))
    wg1_d = din("w_g1T", (COND, IN))
    wb1_d = din("w_b1T", (COND, IN))
    wg2_d = din("w_g2T", (COND, OUT))
    wb2_d = din("w_b2T", (COND, OUT))
    bg1_d = din("b_g1", (IN,))
    bb1_d = din("b_b1", (IN,))
    bg2_d = din("b_g2", (OUT,))
    bb2_d = din("b_b2", (OUT,))
    w1s_d = din("w1sub", (4, 4, IN, OUT))     # [q][tap][ic][o]
    w2t_d = din("w2taps", (9, OUT, OUT))      # [tap][ic][o]
    wsc_d = din("w_scT", (IN, OUT))
    bsc_d = din("b_sc", (OUT,))
    bn1w_d = din("bn1_w", (IN,))
    bn1b_d = din("bn1_b", (IN,))
    bn2w_d = din("bn2_w", (OUT,))
    bn2b_d = din("bn2_b", (OUT,))

    out_d = nc.dram_tensor("out", [B_LOC, OUT, P2], F32, kind="ExternalOutput")

    KT1 = IN // 128
    KT2 = OUT // 128
    MT = TD // 128
    NCH1 = P1 // CHUNK   # 2
    NCH2 = P2 // CHUNK   # 8

    with tile.TileContext(nc) as tc:
        st = contextlib.ExitStack()
        cpool = st.enter_context(tc.tile_pool(name="cpool", bufs=1))
        stg = st.enter_context(tc.tile_pool(name="stg", bufs=2))
        scr = st.enter_context(tc.tile_pool(name="scr", bufs=2))
        psum = st.enter_context(tc.tile_pool(name="psum", bufs=1, space="PSUM"))
        dram = st.enter_context(tc.tile_pool(name="dram", bufs=1, space="DRAM"))
        # phase pools (closed after stage1+conv1 to free SBUF for stage2)
        s1 = contextlib.ExitStack()
        ph1 = s1.enter_context(tc.tile_pool(name="ph1", bufs=1))
        ph1b = s1.enter_context(tc.tile_pool(name="ph1b", bufs=2))

        # ---------------- constants ----------------
        ones = cpool.tile([128, 1], F32R, name="ones")
        onesf = cpool.tile([128, 1], F32, name="onesf")
        nc.vector.memset(onesf[:], 1.0)
        nc.vector.tensor_copy(ones[:], onesf[:])
        onesrow = cpool.tile([1, L], F32R, name="onesrow")
        nc.vector.tensor_copy(onesrow[:], onesf[0:1, 0:1].broadcast_to((1, L)))
        zrow = cpool.tile([128, 1], F32, name="zrow")
        nc.vector.memset(zrow[:], 0.0)
        eps_t = cpool.tile([128, 1], F32, name="eps_t")
        nc.vector.memset(eps_t[:], float(EPS_BN))

        # ================= pass 0: BN1 partial stats (stream x) ==============
        ar1_in = dram.tile([KT1, 128, 2], F32, name="ar1_in")
        ar1_out = dram.tile([KT1, 128, 2], F32, name="ar1_out",
                            addr_space="Shared" if use_collectives else "Local")
        sumP = cpool.tile([128, 4 * KT1], F32, name="sumP")
        ssP = cpool.tile([128, 4 * KT1], F32, name="ssP")
        st1 = cpool.tile([128, 2 * KT1], F32, name="st1")
        for k in range(KT1):
            for c in range(4):   # b_loc * nch1 chunks of 512
                b, n = c // NCH1, c % NCH1
                xin = scr.tile([128, CHUNK], F32, name=f"x0_{k}_{c}", tag="xin", bufs=2)
                nc.sync.dma_start(xin[:], x_d.ap()[b, k * 128:(k + 1) * 128, n * CHUNK:(n + 1) * CHUNK])
                nc.vector.tensor_reduce(sumP[:, 4 * k + c:4 * k + c + 1], xin[:], AX.X, OP.add)
                thr = scr.tile([128, CHUNK], F32, name=f"thr0_{k}_{c}", tag="sq_throw", bufs=1)
                nc.vector.scalar_tensor_tensor(
                    thr[:], xin[:], 0.0, xin[:], OP.add, OP.mult,
                    accum_out=ssP[:, 4 * k + c:4 * k + c + 1])
            nc.vector.tensor_reduce(st1[:, 2 * k:2 * k + 1],
                                    sumP[:, 4 * k:4 * k + 4], AX.X, OP.add)
            nc.vector.tensor_reduce(st1[:, 2 * k + 1:2 * k + 2],
                                    ssP[:, 4 * k:4 * k + 4], AX.X, OP.add)
            nc.sync.dma_start(ar1_in[k], st1[:, 2 * k:2 * k + 2])
        if use_collectives:
            nc.gpsimd.collective_compute(
                "AllReduce", OP.add, replica_groups=[list(range(num_devices))],
                ins=[ar1_in.opt()], outs=[ar1_out.opt()])
        else:
            nc.sync.dma_start(ar1_out[:], ar1_in[:])


        def load_cast(pool, dram_ap, shape, name):
            t = pool.tile(shape, F32R, name=name)
            s = stg.tile(shape, F32, name=f"st_{name}", tag="stage")
            nc.sync.dma_start(s[:], dram_ap)
            nc.vector.tensor_copy(t[:], s[:])
            return t

        def load_vec(dram_t, c, name):
            kt = c // 128
            t = cpool.tile([128, kt], F32, name=name)
            nc.sync.dma_start(t[:], dram_t.ap().rearrange("(k p) -> p k", p=128))
            return t

        bg1 = load_vec(bg1_d, IN, "bg1")
        bb1 = load_vec(bb1_d, IN, "bb1")
        bg2 = load_vec(bg2_d, OUT, "bg2")
        bb2 = load_vec(bb2_d, OUT, "bb2")
        bsc = load_vec(bsc_d, OUT, "bsc")
        bn1w = load_vec(bn1w_d, IN, "bn1w")
        bn1b = load_vec(bn1b_d, IN, "bn1b")
        bn2w = load_vec(bn2w_d, OUT, "bn2w")
        bn2b = load_vec(bn2b_d, OUT, "bn2b")

        words_sb = [[load_cast(cpool, words_d.ap()[b, k * 128:(k + 1) * 128, :], [128, L], f"words_{b}_{k}")
                     for k in range(MT)] for b in range(B_LOC)]
        wordsT_sb = [load_cast(cpool, wordsT_d.ap()[b], [L, TD], f"wordsT_{b}") for b in range(B_LOC)]
        mask_sb = cpool.tile([L, B_LOC], F32, name="mask_sb")
        maskr = cpool.tile([L, B_LOC], F32R, name="maskr")
        mw_sb = cpool.tile([L, B_LOC], F32, name="mw_sb")
        for b in range(B_LOC):
            nc.sync.dma_start(mask_sb[:, b:b + 1], mask_d.ap()[b])
            nc.vector.tensor_copy(maskr[:, b:b + 1], mask_sb[:, b:b + 1])
        gc_sb = [load_cast(cpool, gc_d.ap()[k * 128:(k + 1) * 128, :], [128, B_LOC], f"gc_{k}") for k in range(GD // 128)]

        # stage-1 weights (phase-1 pool)
        wimg1 = [load_cast(ph1, wimg1_d.ap()[k * 128:(k + 1) * 128, :], [128, TD], f"wimg1_{k}") for k in range(KT1)]
        wsc = [load_cast(ph1, wsc_d.ap()[k * 128:(k + 1) * 128, :], [128, OUT], f"wsc_{k}") for k in range(KT1)]
        wg1 = [load_cast(ph1, wg1_d.ap()[k * 128:(k + 1) * 128, :], [128, IN], f"wg1_{k}") for k in range(COND // 128)]
        wb1 = [load_cast(ph1, wb1_d.ap()[k * 128:(k + 1) * 128, :], [128, IN], f"wb1_{k}") for k in range(COND // 128)]

        # ---------------- matvec A/B ----------------
        def matvec_AB(wg, wb, bgv, bbv, cout, name):
            mt = cout // 128
            A = cpool.tile([128, mt * B_LOC], F32, name=f"A_{name}")
            Bv = cpool.tile([128, mt * B_LOC], F32, name=f"B_{name}")
            for m in range(mt):
                pa = psum.tile([128, B_LOC], F32, name=f"pa_{name}_{m}", tag="ps_att", bufs=1)
                for k in range(GD // 128):
                    nc.tensor.matmul(pa[:], wg[k][:, m * 128:(m + 1) * 128], gc_sb[k][:],
                                     start=(k == 0), stop=(k == GD // 128 - 1))
                nc.scalar.activation(A[:, m * B_LOC:(m + 1) * B_LOC], pa[:], ACT.Identity,
                                     bias=bgv[:, m:m + 1], scale=1.0)
                pb = psum.tile([128, B_LOC], F32, name=f"pb_{name}_{m}", tag="ps_att", bufs=1)
                for k in range(GD // 128):
                    nc.tensor.matmul(pb[:], wb[k][:, m * 128:(m + 1) * 128], gc_sb[k][:],
                                     start=(k == 0), stop=(k == GD // 128 - 1))
                nc.scalar.activation(Bv[:, m * B_LOC:(m + 1) * B_LOC], pb[:], ACT.Identity,
                                     bias=bbv[:, m:m + 1], scale=1.0)
            return A, Bv

        A1, B1 = matvec_AB(wg1, wb1, bg1, bb1, IN, "1")

        # words norms
        invwn = []
        for b in range(B_LOC):
            wf = wordsT_sb[b][:].bitcast(F32)
            wsq = scr.tile([L, TD], F32, name=f"wsq_{b}", tag="scr_wsq", bufs=1)
            n2 = cpool.tile([L, 3], F32, name=f"wn2_{b}")
            nc.vector.scalar_tensor_tensor(wsq[:], wf, 0.0, wf, OP.add, OP.mult,
                                           accum_out=n2[:, 0:1])
            nc.scalar.activation(n2[:, 1:2], n2[:, 0:1], ACT.Sqrt, bias=0.0, scale=1.0)
            nc.vector.reciprocal(n2[:, 2:3], n2[:, 1:2])
            nc.vector.tensor_tensor(mw_sb[:, b:b + 1], mask_sb[:, b:b + 1],
                                    n2[:, 2:3], OP.mult)
            invwn.append(n2)

        def bn_post(ar_out_tile, kt, n_total, bnw, bnb, name):
            g = cpool.tile([128, 2 * kt], F32, name=f"g_{name}")
            s_t = cpool.tile([128, kt], F32, name=f"s_{name}")
            t_t = cpool.tile([128, kt], F32, name=f"t_{name}")
            tmp = cpool.tile([128, 4 * kt], F32, name=f"tmp_{name}")
            for k in range(kt):
                nc.sync.dma_start(g[:, 2 * k:2 * k + 2], ar_out_tile[k])
                mean = tmp[:, 4 * k:4 * k + 1]
                var = tmp[:, 4 * k + 1:4 * k + 2]
                std = tmp[:, 4 * k + 2:4 * k + 3]
                istd = tmp[:, 4 * k + 3:4 * k + 4]
                nc.vector.tensor_scalar_mul(mean, g[:, 2 * k:2 * k + 1], 1.0 / n_total)
                nc.vector.scalar_tensor_tensor(var, mean, 0.0, mean, OP.add, OP.mult)
                nc.vector.scalar_tensor_tensor(var, g[:, 2 * k + 1:2 * k + 2], 1.0 / n_total,
                                               var, OP.mult, OP.subtract)
                nc.scalar.activation(std, var, ACT.Sqrt, bias=eps_t[:], scale=1.0)
                nc.vector.reciprocal(istd, std)
                nc.vector.tensor_tensor(s_t[:, k:k + 1], istd, bnw[:, k:k + 1], OP.mult)
                nc.vector.tensor_tensor(t_t[:, k:k + 1], mean, s_t[:, k:k + 1], OP.mult)
                nc.vector.tensor_tensor(t_t[:, k:k + 1], bnb[:, k:k + 1], t_t[:, k:k + 1], OP.subtract)
            return s_t, t_t

        B_STATS = B if use_collectives else B_LOC
        s1v, t1v = bn_post(ar1_out, KT1, B_STATS * P1, bn1w, bn1b, "bn1")

        # ================= generic stage =================
        def stage(b, nch, kt_in, load_src, wimg, wg, wb, A, Bv, s_v, t_v,
                  relu_sink, extra, name):
            """load_src(n) -> (xr_aps, xf_aps): kt_in f32r matmul operands and
            f32 views for the same chunk. extra(n, xr_aps) emits extra matmuls
            (stage1 shortcut)."""
            mt_out = A.shape[1] // B_LOC
            order = list(range(nch))
            if nch == 8:   # stage2 planar chunks: all h=0 planes first so
                order = [0, 2, 4, 6, 1, 3, 5, 7]  # conv2 lower half unblocks early
            for n in order:
                xr, xf = load_src(n)
                if extra is not None:
                    extra(n, xr)
                imgc = []
                for m in range(MT):
                    pim = psum.tile([128, CHUNK], F32, name=f"pim_{name}_{b}_{m}_{n}", tag="ps_mm", bufs=4)
                    for k in range(kt_in):
                        nc.tensor.matmul(pim[:], wimg[k][:, m * 128:(m + 1) * 128],
                                         xr[k], start=(k == 0), stop=(k == kt_in - 1))
                    ic = scr.tile([128, CHUNK], F32R, name=f"imgc_{name}_{b}_{m}_{n}", tag="scr_img", bufs=3)
                    nc.scalar.copy(ic[:], pim[:])
                    imgc.append(ic)
                pn2 = psum.tile([1, CHUNK], F32, name=f"pn2_{name}_{b}_{n}", tag="ps_n2d", bufs=1)
                for m in range(MT):
                    sq = scr.tile([128, CHUNK], F32R, name=f"sqi_{name}_{b}_{m}_{n}", tag="scr_sq", bufs=2)
                    imf = imgc[m][:].bitcast(F32)
                    nc.vector.tensor_tensor(sq[:], imf, imf, OP.mult)
                    nc.tensor.matmul(pn2[:], ones[:], sq[:], start=(m == 0), stop=(m == MT - 1))
                invn = scr.tile([1, 2 * CHUNK], F32, name=f"invn_{name}_{b}_{n}", tag="scr_invn", bufs=1)
                nc.scalar.activation(invn[:, 0:CHUNK], pn2[:], ACT.Sqrt, bias=0.0, scale=1.0)
                nc.vector.reciprocal(invn[:, CHUNK:2 * CHUNK], invn[:, 0:CHUNK])
                invn_r = scr.tile([1, CHUNK], F32R, name=f"invnr_{name}_{b}_{n}", tag="scr_invnr", bufs=1)
                nc.vector.tensor_copy(invn_r[:], invn[:, CHUNK:2 * CHUNK])
                invn_b = psum.tile([L, CHUNK], F32, name=f"invnb_{name}_{b}_{n}", tag="ps_bc", bufs=1)
                nc.tensor.matmul(invn_b[:], onesrow[:], invn_r[:], start=True, stop=True)

                psim = psum.tile([L, CHUNK], F32, name=f"psim_{name}_{b}_{n}", tag="ps_att", bufs=1)
                for m in range(MT):
                    nc.tensor.matmul(psim[:], words_sb[b][m][:], imgc[m][:],
                                     start=(m == 0), stop=(m == MT - 1))
                sim_s = scr.tile([L, CHUNK], F32, name=f"sims_{name}_{b}_{n}", tag="scr_sims", bufs=3)
                nc.scalar.copy(sim_s[:], psim[:])
                tsim = scr.tile([L, CHUNK], F32, name=f"tsim_{name}_{b}_{n}", tag="scr_tsim", bufs=3)
                nc.vector.tensor_tensor(tsim[:], invn_b[:], sim_s[:], OP.mult)
                e_t = scr.tile([L, CHUNK], F32R, name=f"e_{name}_{b}_{n}", tag="scr_e", bufs=3)
                nc.scalar.activation(e_t[:], tsim[:], ACT.Exp, bias=0.0, scale=invwn[b][:, 2:3])
                pden = psum.tile([1, CHUNK], F32, name=f"pden_{name}_{b}_{n}", tag="ps_n2d", bufs=1)
                nc.tensor.matmul(pden[:], maskr[:, b:b + 1], e_t[:], start=True, stop=True)
                rden = scr.tile([1, CHUNK], F32, name=f"rden_{name}_{b}_{n}", tag="scr_rden", bufs=1)
                nc.vector.reciprocal(rden[:], pden[:])
                rden_r = scr.tile([1, CHUNK], F32R, name=f"rdenr_{name}_{b}_{n}", tag="scr_rdenr", bufs=1)
                nc.vector.tensor_copy(rden_r[:], rden[:])
                rden_b = psum.tile([L, CHUNK], F32, name=f"rdenb_{name}_{b}_{n}", tag="ps_bc", bufs=1)
                nc.tensor.matmul(rden_b[:], onesrow[:], rden_r[:], start=True, stop=True)
                en2 = scr.tile([L, CHUNK], F32R, name=f"en2_{name}_{b}_{n}", tag="scr_en2", bufs=3)
                nc.vector.scalar_tensor_tensor(en2[:], rden_b[:], mw_sb[:, b:b + 1],
                                               e_t[:].bitcast(F32), OP.mult, OP.mult)
                ctx = scr.tile([128, MT * CHUNK], F32R, name=f"ctx_{name}_{b}_{n}", tag="scr_ctx")
                for m in range(MT):
                    pctx = psum.tile([128, CHUNK], F32, name=f"pctx_{name}_{b}_{m}_{n}", tag="ps_mm", bufs=4)
                    nc.tensor.matmul(pctx[:], wordsT_sb[b][:, m * 128:(m + 1) * 128], en2[:],
                                     start=True, stop=True)
                    nc.scalar.copy(ctx[:, m * CHUNK:(m + 1) * CHUNK], pctx[:])

                for m in range(mt_out):
                    pg = psum.tile([128, CHUNK], F32, name=f"pg_{name}_{b}_{m}_{n}", tag="ps_mm", bufs=4)
                    for k in range(MT):
                        nc.tensor.matmul(pg[:], wg[2 + k][:, m * 128:(m + 1) * 128],
                                         ctx[:, k * CHUNK:(k + 1) * CHUNK],
                                         start=(k == 0), stop=(k == MT - 1))
                    pb = psum.tile([128, CHUNK], F32, name=f"pb_{name}_{b}_{m}_{n}", tag="ps_mm", bufs=4)
                    for k in range(MT):
                        nc.tensor.matmul(pb[:], wb[2 + k][:, m * 128:(m + 1) * 128],
                                         ctx[:, k * CHUNK:(k + 1) * CHUNK],
                                         start=(k == 0), stop=(k == MT - 1))
                    bnx = scr.tile([128, CHUNK], F32, name=f"bnx_{name}_{b}_{m}_{n}", tag="scr_bnx", bufs=2)
                    nc.vector.scalar_tensor_tensor(
                        bnx[:], xf[m], s_v[:, m:m + 1],
                        t_v[:, m:m + 1].broadcast_to((128, CHUNK)),
                        OP.mult, OP.add)
                    t1 = scr.tile([128, CHUNK], F32, name=f"t1_{name}_{b}_{m}_{n}", tag="scr_t1", bufs=2)
                    nc.vector.scalar_tensor_tensor(
                        t1[:], pg[:], A[:, m * B_LOC + b:m * B_LOC + b + 1],
                        bnx[:], OP.add, OP.mult)
                    pre = scr.tile([128, CHUNK], F32, name=f"pre_{name}_{b}_{m}_{n}", tag="scr_pre", bufs=2)
                    nc.vector.scalar_tensor_tensor(
                        pre[:], pb[:], Bv[:, m * B_LOC + b:m * B_LOC + b + 1],
                        t1[:], OP.add, OP.add)
                    relu_sink(m, n, pre[:])

        # ================= stage 1 + conv1 =================
        PW1 = 34
        out1_d = dram.tile([B_LOC, OUT, P2], F32R, name="out1_spill")
        sum2P = cpool.tile([128, KT2 * B_LOC * 8], F32, name="sum2P")
        ss2P = cpool.tile([128, KT2 * B_LOC * 8], F32, name="ss2P")
        sc_sb = [[cpool.tile([128, P1], F32, name=f"sc_{b}_{m}") for m in range(KT2)]
                 for b in range(B_LOC)]

        for b in range(B_LOC):
            pads = []
            for k in range(KT1):
                pt = ph1b.tile([128, PW1 * PW1], F32R, name=f"pad1_{b}_{k}", tag=f"pad1_{k}", bufs=1)
                nc.vector.tensor_copy(pt[:], zrow[:].broadcast_to((128, PW1 * PW1)))
                pads.append(pt)

            def load1(n, _b=b):
                xr, xf = [], []
                for k in range(KT1):
                    xi = scr.tile([128, CHUNK], F32, name=f"x1_{_b}_{k}_{n}", tag="xin", bufs=2)
                    nc.sync.dma_start(xi[:], x_d.ap()[_b, k * 128:(k + 1) * 128, n * CHUNK:(n + 1) * CHUNK])
                    xc = scr.tile([128, CHUNK], F32R, name=f"xr1_{_b}_{k}_{n}", tag="xr", bufs=5)
                    nc.vector.tensor_copy(xc[:], xi[:])
                    xr.append(xc[:])
                    xf.append(xi[:])
                return xr, xf

            def extra1(n, xr, _b=b):
                for m in range(KT2):
                    ps = psum.tile([128, CHUNK], F32, name=f"psc_{_b}_{m}_{n}", tag="ps_mm", bufs=4)
                    for k in range(KT1):
                        nc.tensor.matmul(ps[:], wsc[k][:, m * 128:(m + 1) * 128], xr[k],
                                         start=(k == 0), stop=(k == KT1 - 1))
                    nc.scalar.activation(sc_sb[_b][m][:, n * CHUNK:(n + 1) * CHUNK], ps[:],
                                         ACT.Identity, bias=bsc[:, m:m + 1], scale=1.0)

            def sink1(m, n, src_ap, _pads=pads):
                v = _pads[m][:].rearrange("p (r c) -> p r c", r=PW1)[:, 1 + 16 * n:1 + 16 * (n + 1), 1:33]
                nc.scalar.activation(v, src_ap.rearrange("p (r c) -> p r c", r=16), ACT.Relu)

            stage(b, NCH1, KT1, load1, wimg1, wg1, wb1, A1, B1, s1v, t1v,
                  sink1, extra1, "s1")

            # conv1 (subpixel) -> spill chunks to DRAM + fused BN2 partials
            for q in range(4):
                a_, b2_ = q // 2, q % 2
                roff = [0, 1] if a_ == 0 else [1, 2]
                coff = [0, 1] if b2_ == 0 else [1, 2]
                w1q = ph1b.tile([128, 4 * KT1 * OUT], F32R, name=f"w1q_{b}_{q}", tag="w1q", bufs=2)
                for ti in range(4):
                    for k in range(KT1):
                        s = stg.tile([128, OUT], F32, name=f"w1st_{b}_{q}_{ti}_{k}", tag="w1stage", bufs=3)
                        nc.sync.dma_start(s[:], w1s_d.ap()[q, ti, k * 128:(k + 1) * 128, :])
                        nc.vector.tensor_copy(
                            w1q[:, (ti * KT1 + k) * OUT:(ti * KT1 + k + 1) * OUT], s[:])
                for m in range(KT2):
                    for n in range(NCH1):
                        pc = psum.tile([128, CHUNK], F32, name=f"pc1_{b}_{q}_{m}_{n}", tag="ps_conv", bufs=1)
                        first = True
                        for ti in range(4):
                            si, tj = ti // 2, ti % 2
                            r0 = 16 * n + roff[si]
                            c0 = coff[tj]
                            for k in range(KT1):
                                rhs = pads[k][:].rearrange("p (r c) -> p r c", r=PW1)[:, r0:r0 + 16, c0:c0 + 32]
                                nc.tensor.matmul(
                                    pc[:],
                                    w1q[:, ((ti * KT1 + k) * OUT + m * 128):((ti * KT1 + k) * OUT + m * 128 + 128)],
                                    rhs, start=first, stop=(ti == 3 and k == KT1 - 1))
                                first = False
                        # planar spill layout: flat = q*1024 + i*32 + j
                        sp = scr.tile([128, CHUNK], F32R, name=f"sp_{b}_{q}_{m}_{n}", tag="spill", bufs=3)
                        ci = (m * B_LOC + b) * 8 + q * NCH1 + n
                        nc.scalar.activation(sp[:], pc[:], ACT.Copy,
                                             accum_out=sum2P[:, ci:ci + 1])
                        thr = scr.tile([128, CHUNK], F32, name=f"thr1_{b}_{q}_{m}_{n}", tag="sq_throw", bufs=1)
                        nc.vector.scalar_tensor_tensor(
                            thr[:], sp[:].bitcast(F32), 0.0, sp[:].bitcast(F32),
                            OP.add, OP.mult, accum_out=ss2P[:, ci:ci + 1])
                        nc.sync.dma_start(
                            out1_d[b, m * 128:(m + 1) * 128,
                                   q * P1 + n * CHUNK: q * P1 + (n + 1) * CHUNK],
                            sp[:])

        # close phase-1 pools
        s1.close()

        # ================= BN2 AllReduce =================
        ar2_in = dram.tile([KT2, 128, 2], F32, name="ar2_in")
        ar2_out = dram.tile([KT2, 128, 2], F32, name="ar2_out",
                            addr_space="Shared" if use_collectives else "Local")
        st2 = cpool.tile([128, 2 * KT2], F32, name="st2")
        for m in range(KT2):
            nc.vector.tensor_reduce(st2[:, 2 * m:2 * m + 1],
                                    sum2P[:, m * 16:(m + 1) * 16], AX.X, OP.add)
            nc.vector.tensor_reduce(st2[:, 2 * m + 1:2 * m + 2],
                                    ss2P[:, m * 16:(m + 1) * 16], AX.X, OP.add)
            nc.sync.dma_start(ar2_in[m], st2[:, 2 * m:2 * m + 2])
        if use_collectives:
            nc.gpsimd.collective_compute(
                "AllReduce", OP.add, replica_groups=[list(range(num_devices))],
                ins=[ar2_in.opt()], outs=[ar2_out.opt()])
        else:
            nc.sync.dma_start(ar2_out[:], ar2_in[:])
        s2v, t2v = bn_post(ar2_out, KT2, B_STATS * P2, bn2w, bn2b, "bn2")

        # ================= stage 2 + conv2 =================
        s2 = contextlib.ExitStack()
        ph2 = s2.enter_context(tc.tile_pool(name="ph2", bufs=1))
        ph2b = s2.enter_context(tc.tile_pool(name="ph2b", bufs=2))

        wimg2 = [load_cast(ph2, wimg2_d.ap()[k * 128:(k + 1) * 128, :], [128, TD], f"wimg2_{k}") for k in range(KT2)]
        wg2 = [load_cast(ph2, wg2_d.ap()[k * 128:(k + 1) * 128, :], [128, OUT], f"wg2_{k}") for k in range(COND // 128)]
        wb2 = [load_cast(ph2, wb2_d.ap()[k * 128:(k + 1) * 128, :], [128, OUT], f"wb2_{k}") for k in range(COND // 128)]
        w2w = [[load_cast(ph2, w2t_d.ap()[t, k * 128:(k + 1) * 128, :], [128, OUT], f"w2_{t}_{k}")
                for k in range(KT2)] for t in range(9)]
        A2, B2 = matvec_AB(wg2, wb2, bg2, bb2, OUT, "2")

        PW2 = 66
        for b in range(B_LOC):
            pads2 = []
            for k in range(KT2):
                pt = ph2b.tile([128, PW2 * PW2], F32R, name=f"pad2_{b}_{k}", tag=f"pad2_{k}", bufs=1)
                nc.vector.tensor_copy(pt[:], zrow[:].broadcast_to((128, PW2 * PW2)))
                pads2.append(pt)

            def load2(n, _b=b):
                xr, xf = [], []
                for k in range(KT2):
                    if SPILL_F32R_DIRECT:
                        oc = scr.tile([128, CHUNK], F32R, name=f"o1_{_b}_{k}_{n}", tag="o1ring", bufs=6)
                        nc.sync.dma_start(oc[:], out1_d[_b, k * 128:(k + 1) * 128, n * CHUNK:(n + 1) * CHUNK])
                        xr.append(oc[:])
                        xf.append(oc[:].bitcast(F32))
                    else:
                        oi = scr.tile([128, CHUNK], F32, name=f"o1f_{_b}_{k}_{n}", tag="o1ring_f", bufs=3)
                        nc.sync.dma_start(oi[:], out1_d[_b, k * 128:(k + 1) * 128, n * CHUNK:(n + 1) * CHUNK].bitcast(F32))
                        oc = scr.tile([128, CHUNK], F32R, name=f"o1_{_b}_{k}_{n}", tag="o1ring", bufs=6)
                        nc.vector.tensor_copy(oc[:], oi[:])
                        xr.append(oc[:])
                        xf.append(oi[:])
                return xr, xf

            def sink2(m, n, src_ap, _pads=pads2):
                # planar chunk n: subpixel plane q = n//2, half h = n%2 holds
                # subgrid rows i in [16h, 16h+16); pad2 coords r = 2i+aq+1,
                # c = 2j+bq+1 (stride-2 interleave de-scatter on the engine)
                qq, hh = n // 2, n % 2
                aq, bq = qq // 2, qq % 2
                r0 = 1 + aq + 32 * hh
                c0 = 1 + bq
                v = _pads[m][:].rearrange("p (r c) -> p r c", r=PW2)[
                    :, r0:r0 + 32:2, c0:c0 + 64:2]
                nc.scalar.activation(v, src_ap.rearrange("p (r c) -> p r c", r=16), ACT.Relu)

            stage(b, NCH2, KT2, load2, wimg2, wg2, wb2, A2, B2, s2v, t2v,
                  sink2, None, "s2")

            for m in range(KT2):
                for n in range(NCH2):
                    pc = psum.tile([128, CHUNK], F32, name=f"pc2_{b}_{m}_{n}", tag="ps_conv", bufs=1)
                    first = True
                    for t in range(9):
                        ku, kv = t // 3, t % 3
                        r0 = 8 * n + ku
                        for k in range(KT2):
                            rhs = pads2[k][:].rearrange("p (r c) -> p r c", r=PW2)[:, r0:r0 + 8, kv:kv + 64]
                            nc.tensor.matmul(pc[:], w2w[t][k][:, m * 128:(m + 1) * 128], rhs,
                                             start=first, stop=(t == 8 and k == KT2 - 1))
                            first = False
                    fin = scr.tile([128, CHUNK], F32, name=f"fin_{b}_{m}_{n}", tag="scr_fin")
                    scv4 = sc_sb[b][m][:].rearrange("p (i j) -> p i j", i=32)[
                        :, 4 * n:4 * n + 4, :].unsqueeze(3).to_broadcast((128, 4, 32, 2))
                    for a_ in (0, 1):
                        nc.vector.tensor_tensor(
                            fin[:].rearrange("p (i a j c) -> p i a j c", i=4, a=2, j=32)[:, :, a_],
                            pc[:].rearrange("p (i a j c) -> p i a j c", i=4, a=2, j=32)[:, :, a_],
                            scv4, OP.add)
                    nc.sync.dma_start(out_d.ap()[b, m * 128:(m + 1) * 128, n * CHUNK:(n + 1) * CHUNK],
                                      fin[:])
        s2.close()
        st.close()

    nc.compile()
    return nc


# ---------------------------------------------------------------------------
# host side
# ---------------------------------------------------------------------------
_cached = {}


def _prep_weights(inputs):
    w = {}
    w["w_img1T"] = inputs["w_img1"].T
    w["w_img2T"] = inputs["w_img2"].T
    w["w_g1T"] = inputs["w_g1"].T
    w["w_b1T"] = inputs["w_b1"].T
    w["w_g2T"] = inputs["w_g2"].T
    w["w_b2T"] = inputs["w_b2"].T
    for k in ("b_g1", "b_b1", "b_g2", "b_b2", "bn1_w", "bn1_b",
              "bn2_w", "bn2_b", "b_sc"):
        w[k] = inputs[k]
    w["w_scT"] = inputs["w_sc"][:, :, 0, 0].T

    wc1 = np.asarray(inputs["w_c1"], np.float32)
    rows = {0: [[0], [1, 2]], 1: [[0, 1], [2]]}
    w1sub = np.zeros((4, 4, IN, OUT), np.float32)
    for a in (0, 1):
        for b2 in (0, 1):
            q = a * 2 + b2
            for si in (0, 1):
                for tj in (0, 1):
                    acc = np.zeros((OUT, IN), np.float32)
                    for ku in rows[a][si]:
                        for kv in rows[b2][tj]:
                            acc += wc1[:, :, ku, kv]
                    w1sub[q, si * 2 + tj] = acc.T
    w["w1sub"] = w1sub
    wc2 = np.asarray(inputs["w_c2"], np.float32)
    w2taps = np.zeros((9, OUT, OUT), np.float32)
    for t in range(9):
        w2taps[t] = wc2[:, :, t // 3, t % 3].T
    w["w2taps"] = w2taps
    return {k: np.ascontiguousarray(v, dtype=np.float32) for k, v in w.items()}


def make_in_maps(inputs):
    w = _prep_weights(inputs)
    x = np.asarray(inputs["x"], np.float32).reshape(B, IN, P1)
    gc = np.asarray(inputs["global_cond"], np.float32)
    words = np.asarray(inputs["words_embs"], np.float32)
    mask01 = (~np.asarray(inputs["mask"])).astype(np.float32)
    in_maps = []
    for c in range(N_CORES):
        sl = slice(c * B_LOC, (c + 1) * B_LOC)
        m = dict(w)
        m["x"] = np.ascontiguousarray(x[sl])
        m["gcT"] = np.ascontiguousarray(gc[sl].T)
        m["words"] = np.ascontiguousarray(words[sl])
        m["wordsT"] = np.ascontiguousarray(words[sl].transpose(0, 2, 1))
        m["mask01"] = np.ascontiguousarray(mask01[sl])
        in_maps.append(m)
    return in_maps


def kernel(**inputs):
    from concourse.bass_utils import run_bass_kernel_spmd
    if "nc" not in _cached:
        _cached["nc"] = build_program()
    nc = _cached["nc"]
    in_maps = make_in_maps(inputs)
    res = run_bass_kernel_spmd(nc, in_maps, core_ids=list(range(N_CORES)))
    out = np.empty((B, OUT, 2 * H, 2 * W), np.float32)
    for c in range(N_CORES):
        out[c * B_LOC:(c + 1) * B_LOC] = res.results[c]["out"].reshape(B_LOC, OUT, 2 * H, 2 * W)
    return out

